# revision 5
# baseline (speedup 1.0000x reference)
"""Trainium2 Bass kernel for nn_BezierGlyph (SIZE=512, 8 strokes x 32 samples).

Math: for every pixel p and curve-sample s_j (256 points):
    d_j = |p - s_j|,  S = sum_j exp(-256 * d_j)
    out = sigmoid(-0.78125 * ln(S) - 8)     (== 1 - coverage of the reference)

Mapping (per core, 32768 pixels):
  - d^2 via TensorE K=4 matmul:  [x,y,1,q] . [-2sx,-2sy,s^2+g,1]  (coords 0.5-centered)
  - d = Sqrt(d^2) on ScalarE (ACT), batched 2048-wide from PSUM
  - e = Exp(-256 d) on ACT  (no max-subtraction needed: exp<=1, underflow -> 0 and
    ln(0+1e-30) -> sigmoid saturates to 1.0 exactly like the f32 reference)
  - per-tile sums on VectorE (segmented 3D reduce)
  - ln+sigmoid epilogue on ACT (tiny), with the +g sqrt-guard compensated in the
    sigmoid bias (-8 - 2500*g)
  ACT work is phased (all sqrts of a half, then all exps) so the sqrt/exp table
  sets load only 5x instead of thrashing per tile.
"""
import numpy as np

SIZE = 512
HW = SIZE * SIZE            # 262144
N_CORES = 8
PXC = HW // N_CORES         # 32768 pixels per core
NPTS = 256                  # 8 strokes * 32 samples
NT = PXC // 128             # 256 tiles of 128 pixels per core
NMEGA = NT // 8             # 32 megas of 8 tiles
SHARP = 256.0               # N_SAMPLES * 8
GUARD = np.float32(5e-6)    # keeps d^2 > 0 against matmul rounding (K=18 psum adds)
# out = sigmoid(200*(m - 0.04)), m = -ln(S)/256; guard shifts m by ~g/(2*0.04)
SIG_SCALE = -200.0 / 256.0
SIG_BIAS = -8.0 - 2500.0 * float(GUARD)

_CACHE = {}


def _build():
    import concourse.bass as bass
    import concourse.mybir as mybir

    nc = bass.Bass()
    f32 = mybir.dt.float32
    bf16 = mybir.dt.bfloat16
    AF = mybir.ActivationFunctionType

    # K=18 bf16 3-way-split of the K=4 fp32 quadratic form: bf16 streams the
    # PE at full rate (even HAM-cold) and FWL halves the weight loads, while
    # the split keeps d^2 error ~1e-7 (plus ~2e-6 of PSUM accumulate rounding,
    # covered by GUARD).
    lt = nc.declare_dram_parameter("lt", [18, PXC], bf16, isOutput=False)
    rh = nc.declare_dram_parameter("rh", [18, NPTS], bf16, isOutput=False)
    out_d = nc.declare_dram_parameter("out", [128, NT], f32, isOutput=True)

    CHUNK = 2048            # lt columns per streamed chunk (16 tiles)
    NCHUNK = PXC // CHUNK   # 16

    with (
        nc.sbuf_tensor([18, CHUNK], bf16) as LTC0,
        nc.sbuf_tensor([18, CHUNK], bf16) as LTC1,
        nc.sbuf_tensor([18, NPTS], bf16) as RH,
        nc.sbuf_tensor([128, 16, 8, NPTS], f32) as D,
        nc.sbuf_tensor([128, 8, NPTS], f32) as E0,
        nc.sbuf_tensor([128, 8, NPTS], f32) as E1,
        nc.sbuf_tensor([128, NT], f32) as SS,
        nc.sbuf_tensor([128, NT], f32) as LNS,
        nc.sbuf_tensor([128, NT], f32) as OUT,
        nc.sbuf_tensor([128, 1], f32) as B_LN,
        nc.sbuf_tensor([128, 1], f32) as B_SIG,
        nc.psum_tensor([128, 8, NPTS], f32) as PA,
        nc.psum_tensor([128, 8, NPTS], f32) as PB,
        nc.semaphore("dma_sem") as dma_sem,
        nc.semaphore("init_sem") as init_sem,
        nc.semaphore("mm_sem") as mm_sem,
        nc.semaphore("sqrt_sem") as sqrt_sem,
        nc.semaphore("exp_sem") as exp_sem,
        nc.semaphore("red_sem") as red_sem,
        nc.semaphore("act_sem") as act_sem,
        nc.Block() as block,
    ):
        EBUF = (E0, E1)
        PSUM = (PA, PB)
        LTC = (LTC0, LTC1)

        @block.gpsimd
        def _(g):
            g.dma_start(RH[0:18, :], rh[:, :]).then_inc(dma_sem, 16)
            g.memset(B_LN[:, :], 1e-30)
            g.memset(B_SIG[:, :], SIG_BIAS).then_inc(init_sem, 1)
            for c in range(NCHUNK):
                if c >= 2:
                    g.wait_ge(mm_sem, 16 * (c - 1))   # PE done with chunk c-2
                g.dma_start(LTC[c % 2][0:18, :], lt[:, CHUNK * c : CHUNK * (c + 1)]
                            ).then_inc(dma_sem, 16)
            g.wait_ge(act_sem, 1)
            g.dma_start(out_d[:, :], OUT[:, :]).then_inc(dma_sem, 16)
            g.wait_ge(dma_sem, (2 + NCHUNK) * 16)

        @block.tensor
        def _(t):
            for M in range(NMEGA):
                if M >= 2:
                    t.wait_ge(sqrt_sem, M - 1)         # psum buffer free
                P = PSUM[M % 2]
                for k in range(8):
                    tile = 8 * M + k
                    c = tile // 16
                    if tile % 16 == 0:
                        t.wait_ge(dma_sem, 16 * (c + 2))   # chunk c resident
                    col = (tile % 16) * 128
                    t.matmul(P[:, k, :], LTC[c % 2][0:18, col : col + 128],
                             RH[0:18, :], start=True, stop=True,
                             tile_position=(0, 0)).then_inc(mm_sem, 1)

        @block.scalar
        def _(s):
            for h in range(2):
                for M in range(16 * h, 16 * h + 16):
                    s.wait_ge(mm_sem, 8 * (M + 1))
                    nc.scalar.activation(D[:, M % 16], PSUM[M % 2][:, :, :],
                                         AF.Sqrt).then_inc(sqrt_sem, 1)
                for M in range(16 * h, 16 * h + 16):
                    if M >= 2:
                        s.wait_ge(red_sem, M - 1)
                    nc.scalar.activation(EBUF[M % 2][:, :, :], D[:, M % 16],
                                         AF.Exp, scale=-SHARP).then_inc(exp_sem, 1)
            s.wait_ge(red_sem, NMEGA)
            s.wait_ge(init_sem, 1)
            nc.scalar.activation(LNS[:, :], SS[:, :], AF.Ln, bias=B_LN[:, :])
            nc.scalar.activation(OUT[:, :], LNS[:, :], AF.Sigmoid,
                                 scale=SIG_SCALE, bias=B_SIG[:, :]
                                 ).then_inc(act_sem, 1)

        @block.vector
        def _(v):
            for M in range(NMEGA):
                v.wait_ge(exp_sem, M + 1)
                nc.vector.tensor_reduce(SS[:, 8 * M : 8 * M + 8],
                                        EBUF[M % 2][:, :, :],
                                        axis=mybir.AxisListType.X,
                                        op=mybir.AluOpType.add
                                        ).then_inc(red_sem, 1)

    return nc


def _bezier_samples(control_points: np.ndarray) -> np.ndarray:
    """(8,4,2) -> (256,2) f32, mirrors the reference's f32 math."""
    pts = np.clip(control_points.astype(np.float32), np.float32(0.0), np.float32(1.0))
    ts = np.linspace(0.0, 1.0, 32).astype(np.float32)
    t = ts[None, :, None]
    mt = np.float32(1.0) - t
    p0, p1, p2, p3 = (pts[:, k : k + 1, :] for k in range(4))
    sam = (mt ** 3 * p0 + np.float32(3.0) * mt ** 2 * t * p1
           + np.float32(3.0) * mt * t ** 2 * p2 + t ** 3 * p3)
    return sam.reshape(-1, 2).astype(np.float32)


def _split_bf3(v64):
    """v (f64) -> 3 bf16 terms summing to v within ~2^-27."""
    import ml_dtypes
    v = np.asarray(v64, np.float64)
    b0 = v.astype(ml_dtypes.bfloat16)
    r = v - b0.astype(np.float64)
    b1 = r.astype(ml_dtypes.bfloat16)
    r2 = r - b1.astype(np.float64)
    b2 = r2.astype(ml_dtypes.bfloat16)
    return b0, b1, b2


def _prep_inputs(control_points: np.ndarray, pixel_grid: np.ndarray):
    import ml_dtypes
    sam = _bezier_samples(np.asarray(control_points))
    sx = (sam[:, 0] - np.float32(0.5)).astype(np.float64)
    sy = (sam[:, 1] - np.float32(0.5)).astype(np.float64)
    ah, am, al = _split_bf3(-2.0 * sx)
    bh, bm, bl = _split_bf3(-2.0 * sy)
    s2h, s2m, s2l = _split_bf3(sx * sx + sy * sy + float(GUARD))
    ones = np.ones(NPTS, ml_dtypes.bfloat16)
    # row pairing (lhsT row k  x  rhs row k):
    #   x: xh*(ah+am+al) + xm*(ah+am) + xl*ah      (6 rows)
    #   y: same                                     (6 rows)
    #   s^2: 1*(s2h+s2m+s2l)                        (3 rows)
    #   q:  (qh+qm+ql)*1                            (3 rows)
    rh = np.ascontiguousarray(np.stack(
        [ah, am, al, ah, am, ah,
         bh, bm, bl, bh, bm, bh,
         s2h, s2m, s2l, ones, ones, ones]).astype(ml_dtypes.bfloat16))

    pg = np.asarray(pixel_grid, dtype=np.float32)
    in_maps = []
    for c in range(N_CORES):
        px = pg[c * PXC : (c + 1) * PXC]
        x = (px[:, 0] - np.float32(0.5)).astype(np.float64)
        y = (px[:, 1] - np.float32(0.5)).astype(np.float64)
        xh, xm, xl = _split_bf3(x)
        yh, ym, yl = _split_bf3(y)
        qh, qm, ql = _split_bf3(x * x + y * y)
        one = np.ones(PXC, ml_dtypes.bfloat16)
        ltv = np.stack([xh, xh, xh, xm, xm, xl,
                        yh, yh, yh, ym, ym, yl,
                        one, one, one, qh, qm, ql])
        in_maps.append({"lt": np.ascontiguousarray(ltv.astype(ml_dtypes.bfloat16)),
                        "rh": rh})
    return in_maps


def _run(inputs, trace=False):
    from concourse.bass_utils import run_bass_kernel_spmd

    if "nc" not in _CACHE:
        _CACHE["nc"] = _build()
    nc = _CACHE["nc"]
    in_maps = _prep_inputs(inputs["control_points"], inputs["pixel_grid"])
    res = run_bass_kernel_spmd(nc, in_maps, core_ids=list(range(N_CORES)),
                               trace=trace)
    parts = []
    for c in range(N_CORES):
        o = res.results[c]["out"]          # (128, NT): [j, t] = pixel 128t+j
        parts.append(np.ascontiguousarray(o.T).reshape(-1))
    full = np.concatenate(parts).reshape(SIZE, SIZE)[None]
    return full.astype(np.float32), res


def kernel(control_points: np.ndarray, pixel_grid: np.ndarray) -> np.ndarray:
    out, _ = _run({"control_points": control_points, "pixel_grid": pixel_grid})
    return out


# revision 7
# speedup vs baseline: 1.0179x; 1.0179x over previous
"""Trainium2 Bass kernel for nn_BezierGlyph (SIZE=512, 8 strokes x 32 samples).

Math: for every pixel p and curve-sample s_j (256 points):
    d_j = |p - s_j|,  S = sum_j exp(-256 * d_j)
    out = sigmoid(-0.78125 * ln(S) - 8)     (== 1 - coverage of the reference)

Mapping (per core, 32768 pixels):
  - d^2 via TensorE K=4 matmul:  [x,y,1,q] . [-2sx,-2sy,s^2+g,1]  (coords 0.5-centered)
  - d = Sqrt(d^2) on ScalarE (ACT), batched 2048-wide from PSUM
  - e = Exp(-256 d) on ACT  (no max-subtraction needed: exp<=1, underflow -> 0 and
    ln(0+1e-30) -> sigmoid saturates to 1.0 exactly like the f32 reference)
  - per-tile sums on VectorE (segmented 3D reduce)
  - ln+sigmoid epilogue on ACT (tiny), with the +g sqrt-guard compensated in the
    sigmoid bias (-8 - 2500*g)
  ACT work is phased (all sqrts of a half, then all exps) so the sqrt/exp table
  sets load only 5x instead of thrashing per tile.
"""
import numpy as np

SIZE = 512
HW = SIZE * SIZE            # 262144
N_CORES = 8
PXC = HW // N_CORES         # 32768 pixels per core
NPTS = 256                  # 8 strokes * 32 samples
NT = PXC // 128             # 256 tiles of 128 pixels per core
NMEGA = NT // 8             # 32 megas of 8 tiles
SHARP = 256.0               # N_SAMPLES * 8
GUARD = np.float32(5e-6)    # keeps d^2 > 0 against matmul rounding (K=18 psum adds)
# out = sigmoid(200*(m - 0.04)), m = -ln(S)/256; guard shifts m by ~g/(2*0.04)
SIG_SCALE = -200.0 / 256.0
SIG_BIAS = -8.0 - 2500.0 * float(GUARD)

_CACHE = {}


def _build():
    import concourse.bass as bass
    import concourse.mybir as mybir

    nc = bass.Bass()
    f32 = mybir.dt.float32
    bf16 = mybir.dt.bfloat16
    AF = mybir.ActivationFunctionType

    # K=18 bf16 3-way-split of the K=4 fp32 quadratic form: bf16 streams the
    # PE at full rate (even HAM-cold) and FWL halves the weight loads, while
    # the split keeps d^2 error ~1e-7 (plus ~2e-6 of PSUM accumulate rounding,
    # covered by GUARD).
    lt = nc.declare_dram_parameter("lt", [18, PXC], bf16, isOutput=False)
    rh = nc.declare_dram_parameter("rh", [18, NPTS], bf16, isOutput=False)
    out_d = nc.declare_dram_parameter("out", [128, NT], f32, isOutput=True)

    CHUNK = 2048            # lt columns per streamed chunk (16 tiles)
    NCHUNK = PXC // CHUNK   # 16

    from contextlib import ExitStack
    with ExitStack() as ctx:
        e = ctx.enter_context
        LTC0 = e(nc.sbuf_tensor([18, CHUNK], bf16))
        LTC1 = e(nc.sbuf_tensor([18, CHUNK], bf16))
        RH = e(nc.sbuf_tensor([18, NPTS], bf16))
        D = e(nc.sbuf_tensor([128, 16, 8, NPTS], f32))
        E0 = e(nc.sbuf_tensor([128, 2, 8, NPTS], f32))
        E1 = e(nc.sbuf_tensor([128, 2, 8, NPTS], f32))
        SS = e(nc.sbuf_tensor([128, NT], f32))
        LNS = e(nc.sbuf_tensor([128, NT], f32))
        OUT = e(nc.sbuf_tensor([128, NT], f32))
        B_WARM = e(nc.sbuf_tensor([128, 1], f32))
        B_LN = e(nc.sbuf_tensor([128, 1], f32))
        B_SIG = e(nc.sbuf_tensor([128, 1], f32))
        PA = e(nc.psum_tensor([128, 8, NPTS], f32))
        PB = e(nc.psum_tensor([128, 8, NPTS], f32))
        dma_sem = e(nc.semaphore("dma_sem"))
        init_sem = e(nc.semaphore("init_sem"))
        mm_sem = e(nc.semaphore("mm_sem"))
        sqrt_sem = e(nc.semaphore("sqrt_sem"))
        exp_sem = e(nc.semaphore("exp_sem"))
        red_sem = e(nc.semaphore("red_sem"))
        act_sem = e(nc.semaphore("act_sem"))
        block = e(nc.Block())

        EBUF = (E0, E1)
        PSUM = (PA, PB)
        LTC = (LTC0, LTC1)
        ONE_AP = nc.const_aps.tensor(1.0, (128, 1))

        @block.gpsimd
        def _(g):
            g.dma_start(RH[0:18, :], rh[:, :]).then_inc(dma_sem, 16)
            g.memset(B_LN[:, :], 1e-30)
            g.memset(B_SIG[:, :], SIG_BIAS).then_inc(init_sem, 1)
            for c in range(NCHUNK):
                if c >= 2:
                    g.wait_ge(mm_sem, 16 * (c - 1))   # PE done with chunk c-2
                g.dma_start(LTC[c % 2][0:18, :], lt[:, CHUNK * c : CHUNK * (c + 1)]
                            ).then_inc(dma_sem, 16)
            g.wait_ge(act_sem, 1)
            g.dma_start(out_d[:, :], OUT[:, :]).then_inc(dma_sem, 16)
            g.wait_ge(dma_sem, (2 + NCHUNK) * 16)

        @block.tensor
        def _(t):
            for M in range(NMEGA):
                if M >= 2:
                    t.wait_ge(sqrt_sem, M - 1)         # psum buffer free
                P = PSUM[M % 2]
                for k in range(8):
                    tile = 8 * M + k
                    c = tile // 16
                    if tile % 16 == 0:
                        t.wait_ge(dma_sem, 16 * (c + 2))   # chunk c resident
                    col = (tile % 16) * 128
                    t.matmul(P[:, k, :], LTC[c % 2][0:18, col : col + 128],
                             RH[0:18, :], start=True, stop=True,
                             tile_position=(0, 0)).then_inc(mm_sem, 1)

        @block.scalar
        def _(s):
            # warm the sqrt table set while input DMAs are in flight
            nc.scalar.activation(B_WARM[:, :], ONE_AP, AF.Sqrt)
            for h in range(2):
                for M in range(16 * h, 16 * h + 16):
                    s.wait_ge(mm_sem, 8 * (M + 1))
                    nc.scalar.activation(D[:, M % 16], PSUM[M % 2][:, :, :],
                                         AF.Sqrt).then_inc(sqrt_sem, 1)
                for j in range(8 * h, 8 * h + 8):      # exp over 2 megas at once
                    if j >= 2:
                        s.wait_ge(red_sem, j - 1)
                    nc.scalar.activation(EBUF[j % 2][:, :, :, :],
                                         D[:, (2 * j) % 16 : (2 * j) % 16 + 2],
                                         AF.Exp, scale=-SHARP).then_inc(exp_sem, 1)
            s.wait_ge(red_sem, NMEGA // 2)
            s.wait_ge(init_sem, 1)
            nc.scalar.activation(LNS[:, :], SS[:, :], AF.Ln, bias=B_LN[:, :])
            nc.scalar.activation(OUT[:, :], LNS[:, :], AF.Sigmoid,
                                 scale=SIG_SCALE, bias=B_SIG[:, :]
                                 ).then_inc(act_sem, 1)

        @block.vector
        def _(v):
            for j in range(NMEGA // 2):
                v.wait_ge(exp_sem, j + 1)
                nc.vector.tensor_reduce(SS[:, 16 * j : 16 * j + 16],
                                        EBUF[j % 2][:, :, :, :],
                                        axis=mybir.AxisListType.X,
                                        op=mybir.AluOpType.add
                                        ).then_inc(red_sem, 1)

    return nc


def _bezier_samples(control_points: np.ndarray) -> np.ndarray:
    """(8,4,2) -> (256,2) f32, mirrors the reference's f32 math."""
    pts = np.clip(control_points.astype(np.float32), np.float32(0.0), np.float32(1.0))
    ts = np.linspace(0.0, 1.0, 32).astype(np.float32)
    t = ts[None, :, None]
    mt = np.float32(1.0) - t
    p0, p1, p2, p3 = (pts[:, k : k + 1, :] for k in range(4))
    sam = (mt ** 3 * p0 + np.float32(3.0) * mt ** 2 * t * p1
           + np.float32(3.0) * mt * t ** 2 * p2 + t ** 3 * p3)
    return sam.reshape(-1, 2).astype(np.float32)


def _split_bf3(v64):
    """v (f64) -> 3 bf16 terms summing to v within ~2^-27."""
    import ml_dtypes
    v = np.asarray(v64, np.float64)
    b0 = v.astype(ml_dtypes.bfloat16)
    r = v - b0.astype(np.float64)
    b1 = r.astype(ml_dtypes.bfloat16)
    r2 = r - b1.astype(np.float64)
    b2 = r2.astype(ml_dtypes.bfloat16)
    return b0, b1, b2


def _prep_inputs(control_points: np.ndarray, pixel_grid: np.ndarray):
    import ml_dtypes
    sam = _bezier_samples(np.asarray(control_points))
    sx = (sam[:, 0] - np.float32(0.5)).astype(np.float64)
    sy = (sam[:, 1] - np.float32(0.5)).astype(np.float64)
    ah, am, al = _split_bf3(-2.0 * sx)
    bh, bm, bl = _split_bf3(-2.0 * sy)
    s2h, s2m, s2l = _split_bf3(sx * sx + sy * sy + float(GUARD))
    ones = np.ones(NPTS, ml_dtypes.bfloat16)
    # row pairing (lhsT row k  x  rhs row k):
    #   x: xh*(ah+am+al) + xm*(ah+am) + xl*ah      (6 rows)
    #   y: same                                     (6 rows)
    #   s^2: 1*(s2h+s2m+s2l)                        (3 rows)
    #   q:  (qh+qm+ql)*1                            (3 rows)
    rh = np.ascontiguousarray(np.stack(
        [ah, am, al, ah, am, ah,
         bh, bm, bl, bh, bm, bh,
         s2h, s2m, s2l, ones, ones, ones]).astype(ml_dtypes.bfloat16))

    pg = np.asarray(pixel_grid, dtype=np.float32)
    in_maps = []
    for c in range(N_CORES):
        px = pg[c * PXC : (c + 1) * PXC]
        x = (px[:, 0] - np.float32(0.5)).astype(np.float64)
        y = (px[:, 1] - np.float32(0.5)).astype(np.float64)
        xh, xm, xl = _split_bf3(x)
        yh, ym, yl = _split_bf3(y)
        qh, qm, ql = _split_bf3(x * x + y * y)
        one = np.ones(PXC, ml_dtypes.bfloat16)
        ltv = np.stack([xh, xh, xh, xm, xm, xl,
                        yh, yh, yh, ym, ym, yl,
                        one, one, one, qh, qm, ql])
        in_maps.append({"lt": np.ascontiguousarray(ltv.astype(ml_dtypes.bfloat16)),
                        "rh": rh})
    return in_maps


def _run(inputs, trace=False):
    from concourse.bass_utils import run_bass_kernel_spmd

    if "nc" not in _CACHE:
        _CACHE["nc"] = _build()
    nc = _CACHE["nc"]
    in_maps = _prep_inputs(inputs["control_points"], inputs["pixel_grid"])
    res = run_bass_kernel_spmd(nc, in_maps, core_ids=list(range(N_CORES)),
                               trace=trace)
    parts = []
    for c in range(N_CORES):
        o = res.results[c]["out"]          # (128, NT): [j, t] = pixel 128t+j
        parts.append(np.ascontiguousarray(o.T).reshape(-1))
    full = np.concatenate(parts).reshape(SIZE, SIZE)[None]
    return full.astype(np.float32), res


def kernel(control_points: np.ndarray, pixel_grid: np.ndarray) -> np.ndarray:
    out, _ = _run({"control_points": control_points, "pixel_grid": pixel_grid})
    return out
